# revision 14
# baseline (speedup 1.0000x reference)
"""Bidirectional attention kernel for Trainium2 (8 NeuronCores, data-parallel over batch).

Math (per batch b):
  sim[l, m] = v1[l] . v2[m]                      (fp32r matmuls, [l,m] layout)
  E_b[l, m] = exp(sim * (1-m1[l]) + (F*m1[l] - C))   fused mask+shift+exp on ACT
              == exp(where(v1_mask[l], F, sim) - C)
  E_a[m, l] = transpose(E_b) * (1-m2[m]) + exp(F-C)*m2[m]   (PE transpose + DVE fix)
              == exp(where(v2_mask[m], F, sim) - C)  up to don't-care rows
  attended_v1[l] = (sum_m E_a[m,l] * v2e[m]) / S_a[l] * keep1[l]   (ones col -> S)
  attended_v2[m] = (sum_l E_b[l,m] * v1e[l]) / S_b[m] * keep2[m]

C is a fixed softmax shift (exp(x-C) instead of exp(x-max)): mathematically
identical softmax, safe because |sim| <~ 91 << 176 and underflow terms are
negligible relative to row sums.

Row-mask (v1_mask) is omitted on the E_a side: softmax over m is row-local and
masked-l rows of attended_v1 are zeroed at the output anyway.
"""
import sys
import types

import ml_dtypes
import numpy as np
from contextlib import ExitStack


def _install_axon_hooks_shim():
    """The image's antenv package lacks the optional axon_hooks module that
    the axon boot shim uses to register the NTFF profiling hook (it degrades
    silently without it). Provide it so trace=True works."""
    if "antenv.axon_hooks" in sys.modules:
        return
    mod = types.ModuleType("antenv.axon_hooks")
    mod._hook = None

    def set_axon_ntff_profile_hook(hook):
        mod._hook = hook

    def get_axon_ntff_profile_hook():
        return mod._hook

    mod.set_axon_ntff_profile_hook = set_axon_ntff_profile_hook
    mod.get_axon_ntff_profile_hook = get_axon_ntff_profile_hook
    sys.modules["antenv.axon_hooks"] = mod
    try:
        import antenv

        antenv.axon_hooks = mod
    except ImportError:
        pass


_install_axon_hooks_shim()

import concourse.bacc as bacc
import concourse.mybir as mybir
import concourse.tile as tile
from concourse.bass_utils import run_bass_kernel_spmd

F32 = mybir.dt.float32
F32R = mybir.dt.float32r
BF16 = mybir.dt.bfloat16
FP16 = mybir.dt.float16
AF = mybir.ActivationFunctionType
ALU = mybir.AluOpType

B, L, D = 64, 1024, 256
NCORES = 8
BPC = B // NCORES          # batches per core
NCH = L // 128             # 8 chunks of 128 along l or m
C_SHIFT = np.float32(88.0)
MASK_FILL = np.float32(-1e-07)


def build_nc():
    nc = bacc.Bacc("TRN2", target_bir_lowering=False, debug=False)

    v1t = nc.dram_tensor("v1t", [BPC, 2, 128, L], FP16, kind="ExternalInput").ap()
    v2t = nc.dram_tensor("v2t", [BPC, 2, 128, L], FP16, kind="ExternalInput").ap()
    v1e = nc.dram_tensor("v1e", [BPC, 128, NCH, D + 2], BF16, kind="ExternalInput").ap()
    v2e = nc.dram_tensor("v2e", [BPC, 128, NCH, D + 2], BF16, kind="ExternalInput").ap()
    mv = nc.dram_tensor("mv", [BPC, 128, 32], F32, kind="ExternalInput").ap()
    ident_d = nc.dram_tensor("ident", [128, 128], BF16, kind="ExternalInput").ap()
    out1 = nc.dram_tensor("out1", [BPC, 128, NCH, D], F32, kind="ExternalOutput").ap()
    out2 = nc.dram_tensor("out2", [BPC, 128, NCH, D], F32, kind="ExternalOutput").ap()

    EW = D + 2  # 258: col 256 = ones (denominator), col 257 = zero pad (fp32r
    # ISA requires even free-dim counts on matmul operands/outputs)

    with tile.TileContext(nc) as tc, ExitStack() as ctx:
        const_pool = ctx.enter_context(tc.tile_pool(name="const", bufs=1))
        in_pool = ctx.enter_context(tc.tile_pool(name="inp", bufs=2))
        e_pool = ctx.enter_context(tc.tile_pool(name="epool", bufs=1))
        out_pool = ctx.enter_context(tc.tile_pool(name="outp", bufs=2))
        small_pool = ctx.enter_context(tc.tile_pool(name="small", bufs=4))
        psb_pool = ctx.enter_context(tc.tile_pool(name="psb", bufs=2, space="PSUM"))
        pst_pool = ctx.enter_context(tc.tile_pool(name="pst", bufs=2, space="PSUM"))
        pso_pool = ctx.enter_context(tc.tile_pool(name="pso", bufs=2, space="PSUM"))

        ident = const_pool.tile([128, 128], BF16)
        nc.sync.dma_start(ident[:], ident_d)

        # PE warmup: ~4us of dummy transposes while the first batch's input
        # DMAs stream in, so the HAM clock-gate is at 2.4 GHz when real
        # matmuls start (and the first sims don't pay the cold-clock ramp).
        warm = pst_pool.tile([128, 512], BF16, tag="psT")
        for w in range(0):
            nc.tensor.transpose(warm[:, (w % 4) * 128:(w % 4 + 1) * 128], ident[:], ident[:])

        # Per-batch state carried across the software-pipelined loop:
        # attend-A of batch b-1 is interleaved into batch b's sim phase so
        # sim matmuls absorb the psO1-recycle latency between attend-A groups.
        prev = None  # (Ea_sb, v2e_sb, mv_sb, out1_sb, b-1)

        def attend_a_group(c, st):
            Ea_p, v2e_p, mv_p, out1_p, bp = st
            psO1 = pso_pool.tile([128, EW], F32, tag="psO")
            for k in range(NCH):
                nc.tensor.matmul(
                    psO1[:],
                    Ea_p[:, k * L + c * 128: k * L + (c + 1) * 128],
                    v2e_p[:, k * EW:(k + 1) * EW],
                    start=(k == 0),
                    stop=(k == NCH - 1),
                )
            rec = small_pool.tile([128, 1], F32, tag="rec1")
            cmb = small_pool.tile([128, 1], F32, tag="cmb1")
            nc.vector.reciprocal(rec[:], psO1[:, D:D + 1])
            # keep1 = sb (cols 0..8)
            nc.vector.tensor_mul(cmb[:], rec[:], mv_p[:, 0 + c: 1 + c])
            nc.vector.tensor_scalar_mul(
                out1_p[:, c * D:(c + 1) * D], psO1[:, 0:D], cmb[:]
            )
            nc.sync.dma_start(out1[bp, :, c], out1_p[:, c * D:(c + 1) * D])

        for b in range(BPC):
            v1t_sb = in_pool.tile([128, 2 * L], FP16, tag="v1t")
            v2t_sb = in_pool.tile([128, 2 * L], FP16, tag="v2t")
            v1e_sb = in_pool.tile([128, NCH * EW], BF16, tag="v1e")
            v2e_sb = in_pool.tile([128, NCH * EW], BF16, tag="v2e")
            mv_sb = in_pool.tile([128, 32], F32, tag="mv")
            nc.sync.dma_start(mv_sb[:], mv[b])
            for k in range(2):
                for h in range(2):
                    sl = slice(k * L + h * 512, k * L + (h + 1) * 512)
                    nc.sync.dma_start(v1t_sb[:, sl], v1t[b, k, :, h * 512:(h + 1) * 512])
                    nc.sync.dma_start(v2t_sb[:, sl], v2t[b, k, :, h * 512:(h + 1) * 512])
            nc.sync.dma_start(v1e_sb[:], v1e[b].rearrange("p c j -> p (c j)"))
            nc.sync.dma_start(v2e_sb[:], v2e[b].rearrange("p c j -> p (c j)"))

            Eb_sb = e_pool.tile([128, NCH * L], BF16, tag="Eb")
            Ea_sb = e_pool.tile([128, NCH * L], BF16, tag="Ea")
            out1_sb = out_pool.tile([128, NCH * D], F32, tag="o1")
            out2_sb = out_pool.tile([128, NCH * D], F32, tag="o2")

            # ---- Phase 1: sim in [l, m] layout + fused mask/shift/exp -> Eb
            # (attend-A groups of batch b-1 interleaved between sim groups)
            for lc in range(NCH):
                psB = psb_pool.tile([128, L], F32, tag="psB")
                for k in range(2):
                    for mh in range(2):
                        nc.tensor.matmul(
                            psB[:, mh * 512:(mh + 1) * 512],
                            v1t_sb[:, k * L + lc * 128: k * L + (lc + 1) * 128],
                            v2t_sb[:, k * L + mh * 512: k * L + mh * 512 + 512],
                            start=(k == 0),
                            stop=(k == 1),
                        )
                nc.scalar.activation(
                    Eb_sb[:, lc * L:(lc + 1) * L],
                    psB[:],
                    AF.Exp,
                    bias=mv_sb[:, 8 + lc: 9 + lc],
                    scale=mv_sb[:, 0 + lc: 1 + lc],
                )
                if prev is not None and lc >= 1:
                    attend_a_group(lc - 1, prev)

            if prev is not None:
                attend_a_group(NCH - 1, prev)

            # ---- Phase 2+3 interleaved: attend B groups between transpose
            # groups so the PSUM->SBUF mask-fix ops (DVE/ACT, ~730ns each)
            # overlap attend-B matmul time on the PE instead of gating it.
            def transp_group(idx):
                g, mc = idx // NCH, idx % NCH
                psT = pst_pool.tile([128, 512], BF16, tag="psT")
                for j in range(4):
                    lc = g * 4 + j
                    nc.tensor.transpose(
                        psT[:, j * 128:(j + 1) * 128],
                        Eb_sb[:, lc * L + mc * 128: lc * L + (mc + 1) * 128],
                        ident[:],
                    )
                # Ea[:, mc*L + g*512 : +512] = psT * sa[mc] + fa[mc]
                dst = Ea_sb[:, mc * L + g * 512: mc * L + g * 512 + 512]
                if idx % 2 == 0:
                    nc.vector.tensor_scalar(
                        dst,
                        psT[:],
                        mv_sb[:, 16 + mc: 17 + mc],
                        mv_sb[:, 24 + mc: 25 + mc],
                        ALU.mult,
                        ALU.add,
                    )
                else:
                    nc.scalar.activation(
                        dst,
                        psT[:],
                        AF.Identity,
                        bias=mv_sb[:, 24 + mc: 25 + mc],
                        scale=mv_sb[:, 16 + mc: 17 + mc],
                    )

            for c in range(NCH):
                psO2 = pso_pool.tile([128, EW], F32, tag="psO")
                for k in range(NCH):
                    nc.tensor.matmul(
                        psO2[:],
                        Eb_sb[:, k * L + c * 128: k * L + (c + 1) * 128],
                        v1e_sb[:, k * EW:(k + 1) * EW],
                        start=(k == 0),
                        stop=(k == NCH - 1),
                    )
                rec = small_pool.tile([128, 1], F32, tag="rec2")
                cmb = small_pool.tile([128, 1], F32, tag="cmb2")
                nc.vector.reciprocal(rec[:], psO2[:, D:D + 1])
                # keep2 = sa (cols 16..24)
                nc.vector.tensor_mul(cmb[:], rec[:], mv_sb[:, 16 + c: 17 + c])
                nc.vector.tensor_scalar_mul(
                    out2_sb[:, c * D:(c + 1) * D], psO2[:, 0:D], cmb[:]
                )
                transp_group(2 * c)
                transp_group(2 * c + 1)
            nc.sync.dma_start(out2[b].rearrange("p c j -> p (c j)"), out2_sb[:])

            prev = (Ea_sb, v2e_sb, mv_sb, out1_sb, b)

        for c in range(NCH):
            attend_a_group(c, prev)

    nc.compile()
    return nc


def _prep_core_inputs(v1c, m1c, v2c, m2c):
    """v1c [BPC, L, D] f32, m1c [BPC, L] bool -> per-core input map."""
    f32 = np.float32
    nb = v1c.shape[0]
    v1t = np.ascontiguousarray(v1c.transpose(0, 2, 1).reshape(nb, 2, 128, L))
    v2t = np.ascontiguousarray(v2c.transpose(0, 2, 1).reshape(nb, 2, 128, L))
    ones = np.ones((nb, L, 1), f32)
    zeros = np.zeros((nb, L, 1), f32)
    bf = ml_dtypes.bfloat16
    v1e = np.concatenate([v1c, ones, zeros], axis=2).reshape(nb, NCH, 128, D + 2)
    v1e = np.ascontiguousarray(v1e.transpose(0, 2, 1, 3)).astype(bf)
    v2e = np.concatenate([v2c, ones, zeros], axis=2).reshape(nb, NCH, 128, D + 2)
    v2e = np.ascontiguousarray(v2e.transpose(0, 2, 1, 3)).astype(bf)

    m1f = m1c.astype(f32)
    m2f = m2c.astype(f32)
    sb = 1.0 - m1f                                   # keep1 / exp scale (side B)
    bb = MASK_FILL * m1f - C_SHIFT                   # exp bias (side B)
    sa = 1.0 - m2f                                   # keep2 / Ea scale (side A)
    fa = np.float32(np.exp(np.float64(MASK_FILL) - np.float64(C_SHIFT))) * m2f
    mv = np.zeros((nb, 128, 32), f32)
    for vec, base in ((sb, 0), (bb, 8), (sa, 16), (fa, 24)):
        mv[:, :, base:base + NCH] = vec.reshape(nb, NCH, 128).transpose(0, 2, 1)
    return {
        "v1t": v1t.astype(np.float16),
        "v2t": v2t.astype(np.float16),
        "v1e": v1e,
        "v2e": v2e,
        "mv": mv,
        "ident": np.eye(128, dtype=ml_dtypes.bfloat16),
    }


def run_on_hw(v1, v1_mask, v2, v2_mask, trace=False, nc=None):
    if nc is None:
        nc = build_nc()
    in_maps = []
    for i in range(NCORES):
        sl = slice(i * BPC, (i + 1) * BPC)
        in_maps.append(_prep_core_inputs(v1[sl], v1_mask[sl], v2[sl], v2_mask[sl]))
    res = run_bass_kernel_spmd(nc, in_maps, core_ids=list(range(NCORES)), trace=trace)
    a1 = np.empty((B, L, D), np.float32)
    a2 = np.empty((B, L, D), np.float32)
    for i, r in enumerate(res.results):
        sl = slice(i * BPC, (i + 1) * BPC)
        a1[sl] = r["out1"].transpose(0, 2, 1, 3).reshape(BPC, L, D)
        a2[sl] = r["out2"].transpose(0, 2, 1, 3).reshape(BPC, L, D)
    return (a1, a2), res


def kernel(v1, v1_mask, v2, v2_mask):
    v1 = np.asarray(v1, np.float32)
    v2 = np.asarray(v2, np.float32)
    v1_mask = np.asarray(v1_mask)
    v2_mask = np.asarray(v2_mask)
    (a1, a2), _ = run_on_hw(v1, v1_mask, v2, v2_mask, trace=False)
    return a1, a2


# revision 15
# speedup vs baseline: 1.0142x; 1.0142x over previous
"""Bidirectional attention kernel for Trainium2 (8 NeuronCores, data-parallel over batch).

Math (per batch b):
  sim[l, m] = v1[l] . v2[m]                      (fp32r matmuls, [l,m] layout)
  E_b[l, m] = exp(sim * (1-m1[l]) + (F*m1[l] - C))   fused mask+shift+exp on ACT
              == exp(where(v1_mask[l], F, sim) - C)
  E_a[m, l] = transpose(E_b) * (1-m2[m]) + exp(F-C)*m2[m]   (PE transpose + DVE fix)
              == exp(where(v2_mask[m], F, sim) - C)  up to don't-care rows
  attended_v1[l] = (sum_m E_a[m,l] * v2e[m]) / S_a[l] * keep1[l]   (ones col -> S)
  attended_v2[m] = (sum_l E_b[l,m] * v1e[l]) / S_b[m] * keep2[m]

C is a fixed softmax shift (exp(x-C) instead of exp(x-max)): mathematically
identical softmax, safe because |sim| <~ 91 << 176 and underflow terms are
negligible relative to row sums.

Row-mask (v1_mask) is omitted on the E_a side: softmax over m is row-local and
masked-l rows of attended_v1 are zeroed at the output anyway.
"""
import sys
import types

import ml_dtypes
import numpy as np
from contextlib import ExitStack


def _install_axon_hooks_shim():
    """The image's antenv package lacks the optional axon_hooks module that
    the axon boot shim uses to register the NTFF profiling hook (it degrades
    silently without it). Provide it so trace=True works."""
    if "antenv.axon_hooks" in sys.modules:
        return
    mod = types.ModuleType("antenv.axon_hooks")
    mod._hook = None

    def set_axon_ntff_profile_hook(hook):
        mod._hook = hook

    def get_axon_ntff_profile_hook():
        return mod._hook

    mod.set_axon_ntff_profile_hook = set_axon_ntff_profile_hook
    mod.get_axon_ntff_profile_hook = get_axon_ntff_profile_hook
    sys.modules["antenv.axon_hooks"] = mod
    try:
        import antenv

        antenv.axon_hooks = mod
    except ImportError:
        pass


_install_axon_hooks_shim()

import concourse.bacc as bacc
import concourse.mybir as mybir
import concourse.tile as tile
from concourse.bass_utils import run_bass_kernel_spmd

F32 = mybir.dt.float32
F32R = mybir.dt.float32r
BF16 = mybir.dt.bfloat16
FP16 = mybir.dt.float16
AF = mybir.ActivationFunctionType
ALU = mybir.AluOpType

B, L, D = 64, 1024, 256
NCORES = 8
BPC = B // NCORES          # batches per core
NCH = L // 128             # 8 chunks of 128 along l or m
C_SHIFT = np.float32(88.0)
MASK_FILL = np.float32(-1e-07)


def build_nc():
    nc = bacc.Bacc("TRN2", target_bir_lowering=False, debug=False)

    v1t = nc.dram_tensor("v1t", [BPC, 2, 128, L], FP16, kind="ExternalInput").ap()
    v2t = nc.dram_tensor("v2t", [BPC, 2, 128, L], FP16, kind="ExternalInput").ap()
    v1e = nc.dram_tensor("v1e", [BPC, 128, NCH, D + 2], BF16, kind="ExternalInput").ap()
    v2e = nc.dram_tensor("v2e", [BPC, 128, NCH, D + 2], BF16, kind="ExternalInput").ap()
    mv = nc.dram_tensor("mv", [BPC, 128, 32], F32, kind="ExternalInput").ap()
    ident_d = nc.dram_tensor("ident", [128, 128], BF16, kind="ExternalInput").ap()
    out1 = nc.dram_tensor("out1", [BPC, 128, NCH, D], F32, kind="ExternalOutput").ap()
    out2 = nc.dram_tensor("out2", [BPC, 128, NCH, D], F32, kind="ExternalOutput").ap()

    EW = D + 2  # 258: col 256 = ones (denominator), col 257 = zero pad (fp32r
    # ISA requires even free-dim counts on matmul operands/outputs)

    with tile.TileContext(nc) as tc, ExitStack() as ctx:
        const_pool = ctx.enter_context(tc.tile_pool(name="const", bufs=1))
        in_pool = ctx.enter_context(tc.tile_pool(name="inp", bufs=2))
        e_pool = ctx.enter_context(tc.tile_pool(name="epool", bufs=1))
        out_pool = ctx.enter_context(tc.tile_pool(name="outp", bufs=2))
        small_pool = ctx.enter_context(tc.tile_pool(name="small", bufs=4))
        psb_pool = ctx.enter_context(tc.tile_pool(name="psb", bufs=2, space="PSUM"))
        pst_pool = ctx.enter_context(tc.tile_pool(name="pst", bufs=2, space="PSUM"))
        pso_pool = ctx.enter_context(tc.tile_pool(name="pso", bufs=2, space="PSUM"))

        ident = const_pool.tile([128, 128], BF16)
        nc.sync.dma_start(ident[:], ident_d)

        # PE warmup: ~4us of dummy transposes while the first batch's input
        # DMAs stream in, so the HAM clock-gate is at 2.4 GHz when real
        # matmuls start (and the first sims don't pay the cold-clock ramp).
        warm = pst_pool.tile([128, 512], BF16, tag="psT")
        for w in range(24):
            nc.tensor.transpose(warm[:, (w % 4) * 128:(w % 4 + 1) * 128], ident[:], ident[:])

        # Per-batch state carried across the software-pipelined loop:
        # attend-A of batch b-1 is interleaved into batch b's sim phase so
        # sim matmuls absorb the psO1-recycle latency between attend-A groups.
        prev = None  # (Ea_sb, v2e_sb, mv_sb, out1_sb, b-1)

        def attend_a_group(c, st):
            Ea_p, v2e_p, mv_p, out1_p, bp = st
            psO1 = pso_pool.tile([128, EW], F32, tag="psO")
            for k in range(NCH):
                nc.tensor.matmul(
                    psO1[:],
                    Ea_p[:, k * L + c * 128: k * L + (c + 1) * 128],
                    v2e_p[:, k * EW:(k + 1) * EW],
                    start=(k == 0),
                    stop=(k == NCH - 1),
                )
            rec = small_pool.tile([128, 1], F32, tag="rec1")
            cmb = small_pool.tile([128, 1], F32, tag="cmb1")
            nc.vector.reciprocal(rec[:], psO1[:, D:D + 1])
            # keep1 = sb (cols 0..8)
            nc.vector.tensor_mul(cmb[:], rec[:], mv_p[:, 0 + c: 1 + c])
            nc.vector.tensor_scalar_mul(
                out1_p[:, c * D:(c + 1) * D], psO1[:, 0:D], cmb[:]
            )
            nc.sync.dma_start(out1[bp, :, c], out1_p[:, c * D:(c + 1) * D])

        for b in range(BPC):
            v1t_sb = in_pool.tile([128, 2 * L], FP16, tag="v1t")
            v2t_sb = in_pool.tile([128, 2 * L], FP16, tag="v2t")
            v1e_sb = in_pool.tile([128, NCH * EW], BF16, tag="v1e")
            v2e_sb = in_pool.tile([128, NCH * EW], BF16, tag="v2e")
            mv_sb = in_pool.tile([128, 32], F32, tag="mv")
            nc.sync.dma_start(mv_sb[:], mv[b])
            for k in range(2):
                for h in range(2):
                    sl = slice(k * L + h * 512, k * L + (h + 1) * 512)
                    nc.sync.dma_start(v1t_sb[:, sl], v1t[b, k, :, h * 512:(h + 1) * 512])
                    nc.sync.dma_start(v2t_sb[:, sl], v2t[b, k, :, h * 512:(h + 1) * 512])
            nc.sync.dma_start(v1e_sb[:], v1e[b].rearrange("p c j -> p (c j)"))
            nc.sync.dma_start(v2e_sb[:], v2e[b].rearrange("p c j -> p (c j)"))

            Eb_sb = e_pool.tile([128, NCH * L], BF16, tag="Eb")
            Ea_sb = e_pool.tile([128, NCH * L], BF16, tag="Ea")
            out1_sb = out_pool.tile([128, NCH * D], F32, tag="o1")
            out2_sb = out_pool.tile([128, NCH * D], F32, tag="o2")

            # ---- Phase 1: sim in [l, m] layout + fused mask/shift/exp -> Eb
            # (attend-A groups of batch b-1 interleaved between sim groups)
            for lc in range(NCH):
                psB = psb_pool.tile([128, L], F32, tag="psB")
                for k in range(2):
                    for mh in range(2):
                        nc.tensor.matmul(
                            psB[:, mh * 512:(mh + 1) * 512],
                            v1t_sb[:, k * L + lc * 128: k * L + (lc + 1) * 128],
                            v2t_sb[:, k * L + mh * 512: k * L + mh * 512 + 512],
                            start=(k == 0),
                            stop=(k == 1),
                        )
                nc.scalar.activation(
                    Eb_sb[:, lc * L:(lc + 1) * L],
                    psB[:],
                    AF.Exp,
                    bias=mv_sb[:, 8 + lc: 9 + lc],
                    scale=mv_sb[:, 0 + lc: 1 + lc],
                )
                if prev is not None and lc >= 1:
                    attend_a_group(lc - 1, prev)

            if prev is not None:
                attend_a_group(NCH - 1, prev)

            # ---- Phase 2+3 interleaved: attend B groups between transpose
            # groups so the PSUM->SBUF mask-fix ops (DVE/ACT, ~730ns each)
            # overlap attend-B matmul time on the PE instead of gating it.
            def transp_group(idx):
                g, mc = idx // NCH, idx % NCH
                psT = pst_pool.tile([128, 512], BF16, tag="psT")
                for j in range(4):
                    lc = g * 4 + j
                    nc.tensor.transpose(
                        psT[:, j * 128:(j + 1) * 128],
                        Eb_sb[:, lc * L + mc * 128: lc * L + (mc + 1) * 128],
                        ident[:],
                    )
                # Ea[:, mc*L + g*512 : +512] = psT * sa[mc] + fa[mc]
                dst = Ea_sb[:, mc * L + g * 512: mc * L + g * 512 + 512]
                if idx % 2 == 0:
                    nc.vector.tensor_scalar(
                        dst,
                        psT[:],
                        mv_sb[:, 16 + mc: 17 + mc],
                        mv_sb[:, 24 + mc: 25 + mc],
                        ALU.mult,
                        ALU.add,
                    )
                else:
                    nc.scalar.activation(
                        dst,
                        psT[:],
                        AF.Identity,
                        bias=mv_sb[:, 24 + mc: 25 + mc],
                        scale=mv_sb[:, 16 + mc: 17 + mc],
                    )

            for c in range(NCH):
                psO2 = pso_pool.tile([128, EW], F32, tag="psO")
                for k in range(NCH):
                    nc.tensor.matmul(
                        psO2[:],
                        Eb_sb[:, k * L + c * 128: k * L + (c + 1) * 128],
                        v1e_sb[:, k * EW:(k + 1) * EW],
                        start=(k == 0),
                        stop=(k == NCH - 1),
                    )
                rec = small_pool.tile([128, 1], F32, tag="rec2")
                cmb = small_pool.tile([128, 1], F32, tag="cmb2")
                nc.vector.reciprocal(rec[:], psO2[:, D:D + 1])
                # keep2 = sa (cols 16..24)
                nc.vector.tensor_mul(cmb[:], rec[:], mv_sb[:, 16 + c: 17 + c])
                nc.vector.tensor_scalar_mul(
                    out2_sb[:, c * D:(c + 1) * D], psO2[:, 0:D], cmb[:]
                )
                transp_group(2 * c)
                transp_group(2 * c + 1)
            nc.sync.dma_start(out2[b].rearrange("p c j -> p (c j)"), out2_sb[:])

            prev = (Ea_sb, v2e_sb, mv_sb, out1_sb, b)

        for c in range(NCH):
            attend_a_group(c, prev)

    nc.compile()
    return nc


def _prep_core_inputs(v1c, m1c, v2c, m2c):
    """v1c [BPC, L, D] f32, m1c [BPC, L] bool -> per-core input map."""
    f32 = np.float32
    nb = v1c.shape[0]
    v1t = np.ascontiguousarray(v1c.transpose(0, 2, 1).reshape(nb, 2, 128, L))
    v2t = np.ascontiguousarray(v2c.transpose(0, 2, 1).reshape(nb, 2, 128, L))
    ones = np.ones((nb, L, 1), f32)
    zeros = np.zeros((nb, L, 1), f32)
    bf = ml_dtypes.bfloat16
    v1e = np.concatenate([v1c, ones, zeros], axis=2).reshape(nb, NCH, 128, D + 2)
    v1e = np.ascontiguousarray(v1e.transpose(0, 2, 1, 3)).astype(bf)
    v2e = np.concatenate([v2c, ones, zeros], axis=2).reshape(nb, NCH, 128, D + 2)
    v2e = np.ascontiguousarray(v2e.transpose(0, 2, 1, 3)).astype(bf)

    m1f = m1c.astype(f32)
    m2f = m2c.astype(f32)
    sb = 1.0 - m1f                                   # keep1 / exp scale (side B)
    bb = MASK_FILL * m1f - C_SHIFT                   # exp bias (side B)
    sa = 1.0 - m2f                                   # keep2 / Ea scale (side A)
    fa = np.float32(np.exp(np.float64(MASK_FILL) - np.float64(C_SHIFT))) * m2f
    mv = np.zeros((nb, 128, 32), f32)
    for vec, base in ((sb, 0), (bb, 8), (sa, 16), (fa, 24)):
        mv[:, :, base:base + NCH] = vec.reshape(nb, NCH, 128).transpose(0, 2, 1)
    return {
        "v1t": v1t.astype(np.float16),
        "v2t": v2t.astype(np.float16),
        "v1e": v1e,
        "v2e": v2e,
        "mv": mv,
        "ident": np.eye(128, dtype=ml_dtypes.bfloat16),
    }


def run_on_hw(v1, v1_mask, v2, v2_mask, trace=False, nc=None):
    if nc is None:
        nc = build_nc()
    in_maps = []
    for i in range(NCORES):
        sl = slice(i * BPC, (i + 1) * BPC)
        in_maps.append(_prep_core_inputs(v1[sl], v1_mask[sl], v2[sl], v2_mask[sl]))
    res = run_bass_kernel_spmd(nc, in_maps, core_ids=list(range(NCORES)), trace=trace)
    a1 = np.empty((B, L, D), np.float32)
    a2 = np.empty((B, L, D), np.float32)
    for i, r in enumerate(res.results):
        sl = slice(i * BPC, (i + 1) * BPC)
        a1[sl] = r["out1"].transpose(0, 2, 1, 3).reshape(BPC, L, D)
        a2[sl] = r["out2"].transpose(0, 2, 1, 3).reshape(BPC, L, D)
    return (a1, a2), res


def kernel(v1, v1_mask, v2, v2_mask):
    v1 = np.asarray(v1, np.float32)
    v2 = np.asarray(v2, np.float32)
    v1_mask = np.asarray(v1_mask)
    v2_mask = np.asarray(v2_mask)
    (a1, a2), _ = run_on_hw(v1, v1_mask, v2, v2_mask, trace=False)
    return a1, a2
